# revision 1
# baseline (speedup 1.0000x reference)
# Multi-head attention on 8 Trainium2 NeuronCores.
#
# Sharding: 8 cores = 4 batches x 2 sequence-halves. Each core receives its
# batch's full x (2048 rows) with its own query-half permuted to the front,
# computes Q for its 1024 rows and K/V for all 2048 keys (softmax over keys is
# permutation invariant), and writes a [1024, 768] slice of the output. No
# collectives.
#
# Per-core pipeline (bf16 matmuls, fp32 accumulation):
#   xT   = transpose(x) via PE                       [768, 2048]
#   qT/kT = (x @ Wq/Wk)^T  via lhsT=W, rhs=xT        [768, 1024/2048]
#   V    = x @ Wv (natural layout, +ones column)     [2048, 12, 65]
#   per head h: S^T = K Q^T  -> exp (ScalarE, scale=1/8, no max subtraction;
#   scores are O(1) so exp cannot overflow fp32) -> P^T staged in SBUF
#   O^T|denom = [V_h | 1] matmul with P^T            [65, 1024]
#   attnT = O^T * (1/denom broadcast via K=1 fp32r matmul)
#   y = attn @ W_out + b_out (lhsT=attnT, rhs=W_out)
#
# Head h's score/exp stage runs interleaved with head h-1's PV stage, so the
# PE->ACT->PE dependency chain has a full head of slack and never stalls the
# in-order engines.
#
# Perf notes (HW-measured via interleaved reps-differencing A/B):
#  - Deep S pipeline (spsum: 4x 1-bank [128,512] tiles, exp per 512 chunk;
#    was 2x 2-bank [128,1024]): the attention phase is bound by the
#    PE->ACT->PE semaphore round-trip, not engine throughput (a static-
#    source exp runs at ~200ns/[128,1024] vs ~2.1us inside the depth-2
#    pipeline). Depth 4 hides the latency: ~160us (ABA 640/668 vs 803).
#  - pvpsum bufs=2 (was 1): head h+1's PV accumulation chain overlaps the
#    DVE reciprocal+mul drain of head h. ~86us (micro: 794->412 ns/MM).
#  - x DMA + transposes emitted before weight DMAs: x gates the transpose ->
#    qkv -> attention chain; weights are only needed later.
#  - Transposes run in bf16 (gpsimd pre-converts x): 1 cyc/row vs 2 for f32.
#  - _build_v2 ("v2"/KVAR=v2) is a flat-PSUM-pool restructure with V and
#    qT/kT computed per head-pair inside the attention loop; measured ~ equal
#    to "full" (kept for reference).
import numpy as np

B, N, D = 4, 2048, 768
H, DH = 12, 64
SCALE = DH ** -0.5
NQ = N // 2          # query rows per core
KT = D // 128        # 6 contraction tiles over D
NKT = N // 128       # 16 key tiles
RT = N // 128        # 16 row tiles of x

_CACHE = {}


def _build_v2(reps=1, sgran=512):
    """Restructured: flat PSUM pools (no phase barrier), V in 3 chunks and
    qT/kTt per head-pair, both interleaved with attention; 4 concurrent
    1-bank PV chains; shared-ones V layout [V_h0 | ones | V_h1] per pair;
    half-sized P^T tiles; bf16 transposes."""
    from concourse import bacc
    import concourse.tile as tile
    import concourse.mybir as mybir

    F32 = mybir.dt.float32
    BF16 = mybir.dt.bfloat16
    AF = mybir.ActivationFunctionType

    nc = bacc.Bacc("TRN2", target_bir_lowering=False, debug=False,
                   num_devices=8)

    x = nc.dram_tensor("x", [N, D], F32, kind="ExternalInput").ap()
    wqkv = nc.dram_tensor("w_qkv", [D, 3 * D], F32, kind="ExternalInput").ap()
    wout = nc.dram_tensor("w_out", [D, D], F32, kind="ExternalInput").ap()
    bout = nc.dram_tensor("b_out", [1, D], F32, kind="ExternalInput").ap()
    ident = nc.dram_tensor("ident", [128, 128], F32, kind="ExternalInput").ap()
    y = nc.dram_tensor("y", [NQ, D], F32, kind="ExternalOutput").ap()

    NP = H // 2          # 6 head pairs
    LAG = 2
    HKT = NKT // 2       # kt tiles per pts half

    with tile.TileContext(nc) as tc:
      for _rep in range(reps):
        with tc.tile_pool(name="const", bufs=1) as const, \
             tc.tile_pool(name="persist", bufs=1) as persist, \
             tc.tile_pool(name="wqk", bufs=1) as wqkp, \
             tc.tile_pool(name="wvx", bufs=1) as wvxp, \
             tc.tile_pool(name="wstage", bufs=1) as wstage, \
             tc.tile_pool(name="xstage", bufs=2) as xstage, \
             tc.tile_pool(name="ktp", bufs=2) as ktp, \
             tc.tile_pool(name="qtp", bufs=2) as qtp, \
             tc.tile_pool(name="ppool", bufs=3) as ppool, \
             tc.tile_pool(name="rpool", bufs=2) as rpool, \
             tc.tile_pool(name="ypool", bufs=2) as ypool, \
             tc.tile_pool(name="qkps", bufs=2, space="PSUM") as qkps, \
             tc.tile_pool(name="spp", bufs=2, space="PSUM") as spp, \
             tc.tile_pool(name="pvp", bufs=4, space="PSUM") as pvp:

            ident_sb = const.tile([128, 128], F32)
            nc.sync.dma_start(out=ident_sb, in_=ident)
            ident_bf = const.tile([128, 128], BF16)
            nc.gpsimd.tensor_copy(out=ident_bf, in_=ident_sb)
            bias_bc = const.tile([128, D], F32)
            nc.gpsimd.dma_start(out=bias_bc, in_=bout.to_broadcast((128, D)))

            wout_bf = persist.tile([128, KT, D], BF16)
            # per pair: [V_h0(64) | ones(64) | V_h1(64)]
            Vni = persist.tile([128, NKT, NP, 192], BF16)
            attnT = persist.tile([128, KT, NQ], BF16)
            nc.vector.memset(Vni[:, :, :, 64:128], 1.0)

            wqk_bf = wqkp.tile([128, KT, 2 * D], BF16)
            wv_bf = wvxp.tile([128, KT, D], BF16)
            xT = wvxp.tile([128, KT, N], BF16)

            for j in range(KT):
                ws = wstage.tile([128, 3 * D], F32, tag="ws", name=f"ws{j}")
                nc.sync.dma_start(out=ws, in_=wqkv[j * 128:(j + 1) * 128, :])
                nc.gpsimd.tensor_copy(out=wqk_bf[:, j, :], in_=ws[:, 0:2 * D])
                nc.gpsimd.tensor_copy(out=wv_bf[:, j, :],
                                      in_=ws[:, 2 * D:3 * D])
            for j in range(KT):
                ws2 = wstage.tile([128, D], F32, tag="ws", name=f"wo{j}")
                nc.sync.dma_start(out=ws2, in_=wout[j * 128:(j + 1) * 128, :])
                nc.gpsimd.tensor_copy(out=wout_bf[:, j, :], in_=ws2)

            for rt in range(RT):
                xs = xstage.tile([128, D], F32, tag="xs", name=f"xs{rt}")
                nc.sync.dma_start(out=xs, in_=x[rt * 128:(rt + 1) * 128, :])
                xb = xstage.tile([128, D], BF16, tag="xb", name=f"xb{rt}")
                nc.gpsimd.tensor_copy(out=xb, in_=xs)
                for g in range(2):
                    tp = qkps.tile([128, 3, 128], BF16, tag="qk",
                                   name=f"tp{rt}_{g}")
                    for jj in range(3):
                        j = g * 3 + jj
                        nc.tensor.transpose(tp[:, jj, :],
                                            xb[:, j * 128:(j + 1) * 128],
                                            ident_bf)
                    nc.vector.tensor_copy(
                        out=xT[:, g * 3:(g + 1) * 3,
                               rt * 128:(rt + 1) * 128],
                        in_=tp)

            def emit_qkT(dst, wcol0, p, rc):
                ps = qkps.tile([128, 512], F32, tag="qk",
                               name=f"qk{wcol0}_{p}_{rc}")
                c0 = wcol0 + p * 128
                for j in range(KT):
                    nc.tensor.matmul(ps, wqk_bf[:, j, c0:c0 + 128],
                                     xT[:, j, rc:rc + 512],
                                     start=(j == 0), stop=(j == KT - 1))
                nc.vector.tensor_copy(out=dst[:, rc:rc + 512], in_=ps)

            def emit_vchunk(c):
                for rt in range(RT):
                    ps = qkps.tile([128, 256], F32, tag="qk",
                                   name=f"v{c}_{rt}")
                    for j in range(KT):
                        nc.tensor.matmul(
                            ps, xT[:, j, rt * 128:(rt + 1) * 128],
                            wv_bf[:, j, c * 256:(c + 1) * 256],
                            start=(j == 0), stop=(j == KT - 1))
                    for pp in range(2):
                        p = 2 * c + pp
                        nc.vector.tensor_copy(
                            out=Vni[:, rt, p, 0:64],
                            in_=ps[:, pp * 128:pp * 128 + 64])
                        nc.vector.tensor_copy(
                            out=Vni[:, rt, p, 128:192],
                            in_=ps[:, pp * 128 + 64:pp * 128 + 128])

            def emit_pair(p, qt_p, kt_p):
                # pts tiles per (head, kt-half): [128, HKT, NQ]
                pts = {}
                pvs = {}

                def emit_s(s, kt):
                    po = 64 * s
                    if kt % HKT == 0:
                        pts[(s, kt // HKT)] = ppool.tile(
                            [128, HKT, NQ], BF16, tag="pt",
                            name=f"pt{p}_{s}_{kt // HKT}")
                    dst = pts[(s, kt // HKT)]
                    for rc in range(0, NQ, sgran):
                        sp = spp.tile([128, sgran], F32, tag="sp",
                                      name=f"sp{p}_{s}_{kt}_{rc}")
                        for r2 in range(0, sgran, 512):
                            nc.tensor.matmul(
                                sp[:, r2:r2 + 512],
                                kt_p[po:po + 64, kt * 128:(kt + 1) * 128],
                                qt_p[po:po + 64, rc + r2:rc + r2 + 512],
                                start=True, stop=True)
                        nc.scalar.activation(
                            dst[:, kt % HKT, rc:rc + sgran], sp,
                            AF.Exp, 0.0, SCALE)

                def emit_pv(s, rc, kt):
                    if kt == 0:
                        pvs[(s, rc)] = pvp.tile([128, 512], F32, tag="pv",
                                                name=f"pv{p}_{s}_{rc}")
                    pv = pvs[(s, rc)]
                    nc.tensor.matmul(
                        pv, Vni[:, kt, p, 64 * s:64 * s + 128],
                        pts[(s, kt // HKT)][:, kt % HKT, rc:rc + 512],
                        start=(kt == 0), stop=(kt == NKT - 1))
                    if kt == NKT - 1:
                        # h0: rows 0-63 = O, 64-127 = denom;
                        # h1 ([ones|V]): rows 0-63 = denom, 64-127 = O
                        od, dd = (0, 64) if s == 0 else (64, 0)
                        rcp = rpool.tile([64, 512], F32, tag="rcp",
                                         name=f"rcp{p}_{s}_{rc}")
                        nc.vector.reciprocal(rcp, pv[dd:dd + 64, :])
                        nc.vector.tensor_mul(
                            attnT[64 * s:64 * s + 64, p, rc:rc + 512],
                            pv[od:od + 64, :], rcp)
                        del pvs[(s, rc)]

                for kt in range(NKT + LAG):
                    if kt < NKT:
                        emit_s(0, kt)
                        emit_s(1, kt)
                    if kt >= LAG:
                        for s in range(2):
                            for rc in (0, 512):
                                emit_pv(s, rc, kt - LAG)

            for c in range(3):
                emit_vchunk(c)
                for pp in range(2):
                    p = 2 * c + pp
                    kt_p = ktp.tile([128, N], BF16, tag="kt", name=f"ktp{p}")
                    qt_p = qtp.tile([128, NQ], BF16, tag="qt", name=f"qtp{p}")
                    for rc in range(0, NQ, 512):
                        emit_qkT(qt_p, 0, p, rc)
                    for rc in range(0, N, 512):
                        emit_qkT(kt_p, D, p, rc)
                    emit_pair(p, qt_p, kt_p)

            # ------------- output projection -------------
            for rt in range(NQ // 128):
                ys = ypool.tile([128, D], F32, tag="ys", name=f"ys{rt}")
                for (c0, cw) in ((0, 512), (512, 256)):
                    yp = spp.tile([128, cw], F32, tag="sp", name=f"yp{rt}_{c0}")
                    for j in range(KT):
                        nc.tensor.matmul(
                            yp,
                            attnT[:, j, rt * 128:(rt + 1) * 128],
                            wout_bf[:, j, c0:c0 + cw],
                            start=(j == 0), stop=(j == KT - 1))
                    nc.vector.tensor_add(ys[:, c0:c0 + cw], yp,
                                         bias_bc[:, c0:c0 + cw])
                nc.sync.dma_start(out=y[rt * 128:(rt + 1) * 128, :], in_=ys)

    nc.compile()
    return nc


def _build(reps=1, variant="full"):
    if ("nc", reps, variant) in _CACHE:
        return _CACHE[("nc", reps, variant)]
    if variant.startswith("v2"):
        nc = _build_v2(reps=reps, sgran=(1024 if variant == "v2w" else 512))
        _CACHE[("nc", reps, variant)] = nc
        return nc

    from concourse import bacc
    import concourse.tile as tile
    import concourse.mybir as mybir

    F32 = mybir.dt.float32
    F32R = mybir.dt.float32r
    BF16 = mybir.dt.bfloat16
    AF = mybir.ActivationFunctionType

    nc = bacc.Bacc("TRN2", target_bir_lowering=False, debug=False,
                   num_devices=8)

    x = nc.dram_tensor("x", [N, D], F32, kind="ExternalInput").ap()
    wqkv = nc.dram_tensor("w_qkv", [D, 3 * D], F32, kind="ExternalInput").ap()
    wout = nc.dram_tensor("w_out", [D, D], F32, kind="ExternalInput").ap()
    bout = nc.dram_tensor("b_out", [1, D], F32, kind="ExternalInput").ap()
    ident = nc.dram_tensor("ident", [128, 128], F32, kind="ExternalInput").ap()
    y = nc.dram_tensor("y", [NQ, D], F32, kind="ExternalOutput").ap()

    with tile.TileContext(nc) as tc:
      for _rep in range(reps):
        with tc.tile_pool(name="const", bufs=1) as const, \
             tc.tile_pool(name="persist", bufs=1) as persist:

            ident_sb = const.tile([128, 128], F32)
            nc.sync.dma_start(out=ident_sb, in_=ident)
            bias_bc = const.tile([128, D], F32)
            nc.gpsimd.dma_start(out=bias_bc, in_=bout.to_broadcast((128, D)))

            wout_bf = persist.tile([128, KT, D], BF16)
            qT = persist.tile([128, KT, NQ], BF16)
            kTt = persist.tile([128, KT, N], BF16)
            Vn = persist.tile([128, NKT, H, 128], BF16)
            attnT = persist.tile([128, KT, NQ], BF16)

            nc.vector.memset(Vn[:, :, :, DH:], 1.0)

            # ------------- phase 1: weights, xT, qT, kT, V -------------
            with tc.tile_pool(name="p1", bufs=1) as p1pool, \
                 tc.tile_pool(name="wstage",
                              bufs=(3 if variant == "deep1" else 2)) as wstage, \
                 tc.tile_pool(name="xstage",
                              bufs=(4 if variant == "deep1" else 3)) as xstage, \
                 tc.tile_pool(name="qkp", bufs=6, space="PSUM") as qkp:
                wqkv_bf = p1pool.tile([128, KT, 3 * D], BF16)
                ident_bf = p1pool.tile([128, 128], BF16)
                nc.gpsimd.tensor_copy(out=ident_bf, in_=ident_sb)

                # x first: it gates transpose -> qkv -> everything.
                xT = p1pool.tile([128, KT, N], BF16)
                for rt in range(RT):
                    xs = xstage.tile([128, D], F32, tag="xs", name=f"xs{rt}")
                    nc.sync.dma_start(out=xs, in_=x[rt * 128:(rt + 1) * 128, :])
                    xb = xstage.tile([128, D], BF16, tag="xb", name=f"xb{rt}")
                    nc.gpsimd.tensor_copy(out=xb, in_=xs)
                    for g in range(2):
                        tp = qkp.tile([128, 3, 128], BF16, tag="qk",
                                      name=f"tp{rt}_{g}")
                        for jj in range(3):
                            j = g * 3 + jj
                            nc.tensor.transpose(tp[:, jj, :],
                                                xb[:, j * 128:(j + 1) * 128],
                                                ident_bf)
                        nc.vector.tensor_copy(
                            out=xT[:, g * 3:(g + 1) * 3,
                                   rt * 128:(rt + 1) * 128],
                            in_=tp)

                for j in range(KT):
                    ws = wstage.tile([128, 3 * D], F32, tag="ws", name=f"ws{j}")
                    nc.sync.dma_start(out=ws, in_=wqkv[j * 128:(j + 1) * 128, :])
                    nc.gpsimd.tensor_copy(out=wqkv_bf[:, j, :], in_=ws)
                for j in range(KT):
                    ws2 = wstage.tile([128, D], F32, tag="ws", name=f"wo{j}")
                    nc.sync.dma_start(out=ws2, in_=wout[j * 128:(j + 1) * 128, :])
                    nc.gpsimd.tensor_copy(out=wout_bf[:, j, :], in_=ws2)

                def emit_qkvT(dst, wcol0, ct, rc):
                    ps = qkp.tile([128, 512], F32, tag="qk",
                                  name=f"qk{wcol0}_{ct}_{rc}")
                    c0 = wcol0 + ct * 128
                    for j in range(KT):
                        nc.tensor.matmul(ps, wqkv_bf[:, j, c0:c0 + 128],
                                         xT[:, j, rc:rc + 512],
                                         start=(j == 0), stop=(j == KT - 1))
                    nc.vector.tensor_copy(out=dst[:, ct, rc:rc + 512], in_=ps)

                for ct in range(KT):
                    for rc in range(0, NQ, 512):
                        emit_qkvT(qT, 0, ct, rc)
                    for rc in range(0, N, 512):
                        emit_qkvT(kTt, D, ct, rc)
                for rt in range(RT):
                    for (c0, cw) in ((0, 512), (512, 256)):
                        ps = qkp.tile([128, 512], F32, tag="qk",
                                      name=f"v{rt}_{c0}")
                        for j in range(KT):
                            nc.tensor.matmul(
                                ps[:, :cw],
                                xT[:, j, rt * 128:(rt + 1) * 128],
                                wqkv_bf[:, j, 2 * D + c0:2 * D + c0 + cw],
                                start=(j == 0), stop=(j == KT - 1))
                        nc.vector.tensor_copy(
                            out=Vn[:, rt, c0 // DH:(c0 + cw) // DH, 0:DH],
                            in_=ps[:, :cw].rearrange("p (h d) -> p h d", d=DH))

            # ------------- phase 2: attention (head-lagged pipeline) ----
            with tc.tile_pool(name="ppool",
                              bufs=(4 if variant == "spair" else 2)) as ppool, \
                 tc.tile_pool(name="rpool", bufs=2) as rpool, \
                 tc.tile_pool(name="ypool", bufs=2) as ypool, \
                 tc.tile_pool(name="spsum",
                              bufs=(3 if variant == "pv1buf" else
                                    2 if variant == "sp2" else 4),
                              space="PSUM") as spsum, \
                 tc.tile_pool(name="pvpsum",
                              bufs=(1 if variant == "pv1buf" else
                                    2 if variant in ("sp2", "pv2") else 4),
                              space="PSUM") as pvpsum:

                PDEPTH = 4 if variant in ("spair", "deep1") else NKT
                pts = {}
                pvs = {}
                ptfix = None
                if variant in ("nx1", "nx2", "pvonly", "pvna", "pv64"):
                    ptfix = ppool.tile([128, NKT, NQ], BF16, tag="ptfix",
                                       bufs=1)
                    nc.vector.memset(ptfix[:, 0, :], 0.001)
                    for kk in range(1, NKT):
                        nc.vector.tensor_copy(out=ptfix[:, kk, :],
                                              in_=ptfix[:, 0, :])

                def emit_s(h, kt):
                    tj, po = divmod(h, 2)
                    po *= 64
                    if kt == 0:
                        pts[h] = ppool.tile([128, NKT, NQ], BF16, tag="pt",
                                            name=f"pt{h}")
                    if variant not in ("sp2", "pv1buf", "wide", "dvexp",
                                       "nx1", "nx2", "sonly"):
                        # deep S pipeline: 1-bank [128,512] tiles, 4 slots;
                        # exp per 512 chunk. Hides the PE->ACT->PE semaphore
                        # round-trip that throttled depth-2 pipelining.
                        for rc in range(0, NQ, 512):
                            sp = spsum.tile([128, 512], F32, tag="sp",
                                            name=f"sp{h}_{kt}_{rc}")
                            nc.tensor.matmul(
                                sp,
                                kTt[po:po + 64, tj, kt * 128:(kt + 1) * 128],
                                qT[po:po + 64, tj, rc:rc + 512],
                                start=True, stop=True)
                            nc.scalar.activation(
                                pts[h][:, kt, rc:rc + 512], sp,
                                AF.Exp, 0.0, SCALE)
                        return
                    sp = spsum.tile([128, NQ], F32, tag="sp",
                                    name=f"sp{h}_{kt}")
                    if variant == "wide":
                        nc.tensor.matmul(
                            sp,
                            kTt[po:po + 64, tj, kt * 128:(kt + 1) * 128],
                            qT[po:po + 64, tj, :],
                            start=True, stop=True)
                    else:
                        for rc in range(0, NQ, 512):
                            nc.tensor.matmul(
                                sp[:, rc:rc + 512],
                                kTt[po:po + 64, tj, kt * 128:(kt + 1) * 128],
                                qT[po:po + 64, tj, rc:rc + 512],
                                start=True, stop=True)
                    if variant == "dvexp":
                        nc.vector.tensor_copy(out=pts[h][:, kt, :], in_=sp)
                    elif variant in ("nx1", "sonly"):
                        pass
                    elif variant == "nx2":
                        nc.scalar.activation(pts[h][:, kt, :], sp, AF.Exp,
                                             0.0, SCALE)
                    else:
                        nc.scalar.activation(pts[h][:, kt, :], sp, AF.Exp,
                                             0.0, SCALE)

                def emit_s2(p, kt):
                    # both heads of pair p, rc-interleaved across row groups;
                    # pts is a rolling PDEPTH-deep buffer (PV lags by PLAG=2,
                    # so only a few kt slices are ever live)
                    for h in (2 * p, 2 * p + 1):
                        if kt == 0:
                            pts[h] = ppool.tile([128, PDEPTH, NQ], BF16,
                                                tag="pt", name=f"pt{h}")
                    for rc in range(0, NQ, 512):
                        for s in range(2):
                            h = 2 * p + s
                            po = 64 * s
                            sp = spsum.tile([128, 512], F32, tag="sp",
                                            name=f"sp{h}_{kt}_{rc}")
                            nc.tensor.matmul(
                                sp,
                                kTt[po:po + 64, p, kt * 128:(kt + 1) * 128],
                                qT[po:po + 64, p, rc:rc + 512],
                                start=True, stop=True)
                            nc.scalar.activation(
                                pts[h][:, kt % PDEPTH, rc:rc + 512], sp,
                                AF.Exp, 0.0, SCALE)

                def emit_pv(h, kt):
                    tj, po = divmod(h, 2)
                    po *= 64
                    if variant not in ("sp2", "pv2", "pv1buf", "wide", "nx1",
                                       "nx2", "pvonly", "pair", "pvna",
                                       "pv64"):
                        # deep PV pipeline: two 1-bank [128,512] accumulation
                        # chains per head, 4 slots -> head h runs while head
                        # h-1's DVE reciprocal/mul drains.
                        if kt == 0:
                            pvs[h] = [pvpsum.tile([128, 512], F32, tag="pv",
                                                  name=f"pv{h}_{rc}")
                                      for rc in (0, 512)]
                        for i, rc in enumerate((0, 512)):
                            nc.tensor.matmul(
                                pvs[h][i], Vn[:, kt, h, :],
                                pts[h][:, kt % PDEPTH, rc:rc + 512],
                                start=(kt == 0), stop=(kt == NKT - 1))
                        if kt == NKT - 1:
                            pts.pop(h, None)
                            for i, rc in enumerate((0, 512)):
                                pv = pvs[h][i]
                                rcp = rpool.tile([64, 512], F32, tag="rcp",
                                                 name=f"rcp{h}_{rc}")
                                nc.vector.reciprocal(rcp, pv[DH:DH + 64, :])
                                nc.vector.tensor_mul(
                                    attnT[po:po + 64, tj, rc:rc + 512],
                                    pv[0:DH, :], rcp)
                            del pvs[h]
                        return
                    if kt == 0:
                        pvs[h] = pvpsum.tile([128, NQ], F32, tag="pv",
                                             name=f"pv{h}")
                    pv = pvs[h]
                    ptsrc = (ptfix if variant in ("nx1", "nx2", "pvonly")
                             else pts[h])
                    if variant == "wide":
                        nc.tensor.matmul(
                            pv, Vn[:, kt, h, :], ptsrc[:, kt, :],
                            start=(kt == 0), stop=(kt == NKT - 1))
                    else:
                        for rc in range(0, NQ, 512):
                            nc.tensor.matmul(
                                pv[:, rc:rc + 512], Vn[:, kt, h, :],
                                ptsrc[:, kt, rc:rc + 512],
                                start=(kt == 0), stop=(kt == NKT - 1))
                    if kt == NKT - 1:
                        pts.pop(h, None)
                        rcp = rpool.tile([64, NQ], F32, tag="rcp",
                                         name=f"rcp{h}")
                        nc.vector.reciprocal(rcp, pv[DH:DH + 64, :])
                        nc.vector.tensor_mul(attnT[po:po + 64, tj, :],
                                             pv[0:DH, :], rcp)
                        del pvs[h]

                if variant == "pair":
                    LAG = 3
                    for p in range(H // 2):
                        h0, h1 = 2 * p, 2 * p + 1
                        for kt in range(NKT + LAG):
                            if kt < NKT:
                                emit_s(h0, kt)
                                emit_s(h1, kt)
                            if kt >= LAG:
                                emit_pv(h0, kt - LAG)
                                emit_pv(h1, kt - LAG)
                elif variant == "noattn":
                    nc.vector.memset(attnT, 0.0)
                elif variant == "sonly":
                    for h in range(H):
                        for kt in range(NKT):
                            emit_s(h, kt)
                        del pts[h]
                    nc.vector.memset(attnT, 0.0)
                elif variant == "pvonly":
                    nc.vector.memset(attnT, 0.0)
                    for h in range(H):
                        for kt in range(NKT):
                            emit_pv(h, kt)
                elif variant == "pvna":
                    # PV matmuls without accumulation chains: independent
                    # start/stop into rotating sp slots
                    nc.vector.memset(attnT, 0.0)
                    for h in range(H):
                        for kt in range(NKT):
                            spx = spsum.tile([128, NQ], F32, tag="sp",
                                             name=f"spx{h}_{kt}")
                            for rc in range(0, NQ, 512):
                                nc.tensor.matmul(
                                    spx[:, rc:rc + 512],
                                    Vn[:, kt, h, :],
                                    ptfix[:, kt, rc:rc + 512],
                                    start=True, stop=True)
                elif variant == "pv64":
                    # accumulating PV with 64-col stationary (no ones col)
                    nc.vector.memset(attnT, 0.0)
                    for h in range(H):
                        pvx = pvpsum.tile([128, NQ], F32, tag="pv",
                                          name=f"pvx{h}")
                        for kt in range(NKT):
                            for rc in range(0, NQ, 512):
                                nc.tensor.matmul(
                                    pvx[0:DH, rc:rc + 512],
                                    Vn[:, kt, h, 0:DH],
                                    ptfix[:, kt, rc:rc + 512],
                                    start=(kt == 0), stop=(kt == NKT - 1))
                elif variant in ("spair", "spairbig", "deep1"):
                    # pair-major: S for both heads of a pair per kt with
                    # rc-interleaved row groups (0-63 / 64-127 alternate) so
                    # consecutive S MMs land in different PE row groups and
                    # overlap; PV lags within the pair.
                    PLAG = 3 if variant == "deep1" else 2
                    for p in range(H // 2):
                        h0, h1 = 2 * p, 2 * p + 1
                        for kt in range(NKT + PLAG):
                            if kt < NKT:
                                emit_s2(p, kt)
                            if kt >= PLAG:
                                emit_pv(h0, kt - PLAG)
                                emit_pv(h1, kt - PLAG)
                else:
                    for h in range(H):
                        for kt in range(NKT):
                            emit_s(h, kt)
                            if h >= 1:
                                emit_pv(h - 1, kt)
                    for kt in range(NKT):
                        emit_pv(H - 1, kt)

                # ------------- phase 3: output projection -------------
                for rt in range(NQ // 128):
                    ys = ypool.tile([128, D], F32, tag="ys", name=f"ys{rt}")
                    for (c0, cw) in ((0, 512), (512, 256)):
                        yp = spsum.tile([128, cw], F32, tag="sp",
                                        name=f"yp{rt}_{c0}")
                        for j in range(KT):
                            nc.tensor.matmul(
                                yp,
                                attnT[:, j, rt * 128:(rt + 1) * 128],
                                wout_bf[:, j, c0:c0 + cw],
                                start=(j == 0), stop=(j == KT - 1))
                        nc.vector.tensor_add(ys[:, c0:c0 + cw], yp,
                                             bias_bc[:, c0:c0 + cw])
                    nc.sync.dma_start(out=y[rt * 128:(rt + 1) * 128, :],
                                      in_=ys)

    nc.compile()
    _CACHE[("nc", reps, variant)] = nc
    return nc


def _in_maps(x, W_qkv, W_out, b_out):
    x = np.ascontiguousarray(np.asarray(x, dtype=np.float32))
    W_qkv = np.ascontiguousarray(np.asarray(W_qkv, dtype=np.float32))
    W_out = np.ascontiguousarray(np.asarray(W_out, dtype=np.float32))
    b_out = np.ascontiguousarray(np.asarray(b_out, dtype=np.float32)).reshape(1, D)
    ident = np.eye(128, dtype=np.float32)
    maps = []
    for c in range(8):
        b, half = divmod(c, 2)
        xb = x[b]
        xr = np.concatenate(
            [xb[half * NQ:(half + 1) * NQ], xb[(1 - half) * NQ:(2 - half) * NQ]],
            axis=0)
        maps.append({"x": np.ascontiguousarray(xr), "w_qkv": W_qkv,
                     "w_out": W_out, "b_out": b_out, "ident": ident})
    return maps


def kernel(x, W_qkv, W_out, b_out):
    import os
    from concourse import bass_utils
    nc = _build(variant=os.environ.get("KVAR", "deep1"))
    maps = _in_maps(x, W_qkv, W_out, b_out)
    res = bass_utils.run_bass_kernel_spmd(nc, maps, core_ids=list(range(8)))
    out = np.empty((B, N, D), dtype=np.float32)
    for c in range(8):
        b, half = divmod(c, 2)
        out[b, half * NQ:(half + 1) * NQ] = res.results[c]["y"]
    return out

